# revision 19
# baseline (speedup 1.0000x reference)
"""Trainium2 Bass kernel for nn_BinaryDense: out = x @ (sum_k sign(b_k)*a_k) + bias.

Shapes (hardcoded): x [4096,4096] f32, b [4,4096,4096] f32, a [4,4096] f32,
bias [4096] f32 -> out [4096,4096] f32.

Strategy: tensor-parallel over the output (units) dim across 8 NeuronCores.
Core c owns O-columns [c*512, (c+1)*512).

Per core:
  1. Build w[:, oc] = sum_k copysign(a[k,oc], b[k,:,oc]) on-chip. b arrives
     bf16 in [I, K, O_c] layout (k-major): copysign is two bitwise DVE ops
     ((b & 0x8000) | a) and the k-sum is two dense bf16 adds.
  2. Hybrid-precision matmul with fp32 PSUM accumulation, contraction split:
     - k-tiles 0..25 (26 tiles) in bf16: lhsT = x^T tiles (host bf16 cast),
       rhs = w tiles. k-blocks [9, 9, 8] with an fp32 SBUF accumulator per
       m-tile (init includes bias). kb1/kb2 acc += psum runs on GpSimd to
       keep DVE on the build.
     - k-tiles 26..31 (3 pairs) in fp8-e4m3 DoubleRow (2x PE throughput):
       lhsT = paired x8^T [128,2,128] (host fp8 cast), rhs = paired w8
       [128,2,512] written directly by the build's final add (DVE converts
       on write). Quantization error of this 6/32 slice measures ~1.7%
       end-to-end, inside the 2e-2 budget.
  3. Finale without DVE: after kb2, out[m] = acc[m] is DMA'd out; after the
     fp8 block's psum stop, out[m] += psum via GpSimd DMA accum (CCE add).

Host side only reshapes/casts/shards (no math): x^T bf16, x8^T e4m3 pair
layout, b -> [I,K,O] bf16, a/bias broadcast rows.
"""

import sys

if "/opt/trn_rl_repo" not in sys.path:
    sys.path.insert(0, "/opt/trn_rl_repo")

import numpy as np
import ml_dtypes

BF16 = ml_dtypes.bfloat16
E4M3 = ml_dtypes.float8_e4m3

B = 4096   # batch rows of x
I = 4096   # input dim (contraction)
O = 4096   # output dim (sharded)
K = 4      # binary bases
NCORES = 8
OC = O // NCORES   # 512 output cols per core
P = 128

KT = I // P        # 32 k-tiles (contraction)
MT = B // P        # 32 m-tiles (output rows)
M_BLOCK = 4        # m-tiles per psum block (4 tags x 2 bufs = 8 banks)

KT_BF = 26         # k-tiles 0..25 in bf16
N_PAIR = 3         # k-tiles 26..31 as fp8 DoubleRow pairs
K_BLOCKS = [9, 9, 8]   # bf16 k-blocks (sum = KT_BF)


def _build_program():
    import concourse.bass as bass
    import concourse.mybir as mybir
    from concourse import bacc
    from concourse.tile import TileContext

    nc = bacc.Bacc(None, target_bir_lowering=False)

    b_re = nc.declare_dram_parameter("b_re", [I, K * OC], mybir.dt.bfloat16, isOutput=False)
    b8_re = nc.declare_dram_parameter("b8_re", [7 * P, K * OC], mybir.dt.float8e4, isOutput=False)
    a_b = nc.declare_dram_parameter("a_b", [P, K * OC], mybir.dt.bfloat16, isOutput=False)
    xT = nc.declare_dram_parameter("xT", [I, B], mybir.dt.bfloat16, isOutput=False)
    x8dr = nc.declare_dram_parameter("x8dr", [N_PAIR * P, 2, B], mybir.dt.float8e4, isOutput=False)
    bias_b = nc.declare_dram_parameter("bias_b", [P, OC], mybir.dt.float32, isOutput=False)
    out = nc.declare_dram_parameter("out", [B, OC], mybir.dt.float32, isOutput=True)

    with TileContext(nc) as tc:
        with (
            tc.tile_pool(name="const", bufs=1) as const,
            tc.tile_pool(name="bpool", bufs=8) as bpool,
            tc.tile_pool(name="b8pool", bufs=7) as b8pool,
            tc.tile_pool(name="cpool", bufs=4) as cpool,
            tc.tile_pool(name="tpool", bufs=4) as tpool,
            tc.tile_pool(name="wpool", bufs=1) as wpool,
            tc.tile_pool(name="w8pool", bufs=1) as w8pool,
            tc.tile_pool(name="xpool", bufs=12) as xpool,
            tc.tile_pool(name="x8pool", bufs=6) as x8pool,
            tc.tile_pool(name="apool", bufs=1) as apool,
            tc.tile_pool(name="o8pool", bufs=4) as o8pool,
            tc.tile_pool(name="eapool", bufs=4) as eapool,
            tc.tile_pool(name="psum", bufs=2, space="PSUM") as psum_pool,
        ):
            # consts on SWDGE so the HWDGE queue starts with b0/xt0
            a_tile = const.tile([P, K * OC], mybir.dt.bfloat16)
            nc.gpsimd.dma_start(out=a_tile[:], in_=a_b[:, :])
            bias_tile = const.tile([P, OC], mybir.dt.float32)
            nc.gpsimd.dma_start(out=bias_tile[:], in_=bias_b[:, :])
            mask_tile = const.tile([P, 1], mybir.dt.int32)
            nc.vector.memset(mask_tile[:], -2147450880)  # 0x80008000: bf16 sign pair

            # ---- w build: per k-tile [P, OC]; tiles >= KT_BF write fp8 pairs ----
            b_tiles_live = {}
            contrib_live = {}
            t_live = {}
            w_tiles = [None] * KT_BF
            w8_tiles = [None] * N_PAIR
            for g in range(N_PAIR):
                w8_tiles[g] = w8pool.tile([P, 2, OC], mybir.dt.float8e4, name=f"w8_{g}")

            def emit_dma(kt, queue=None):
                # startup tiles ride sync (critical path); mid-kernel tiles
                # ride gpsimd so they don't head-of-line block the xt stream
                b_tile = bpool.tile([P, K * OC], mybir.dt.bfloat16, name="b_tile")
                (queue or nc.gpsimd).dma_start(out=b_tile[:], in_=b_re[kt * P:(kt + 1) * P, :])
                b_tiles_live[kt] = b_tile

            def emit_dma8(kt):
                # startup k-tiles ship as fp8 (sign-exact, half the DMA bytes)
                # and convert to bf16 on the idle ACT engine
                b8_tile = b8pool.tile([P, K * OC], mybir.dt.float8e4, name="b8_tile")
                nc.sync.dma_start(out=b8_tile[:], in_=b8_re[kt * P:(kt + 1) * P, :])
                b_tile = bpool.tile([P, K * OC], mybir.dt.bfloat16, name="b_tile")
                nc.scalar.copy(out=b_tile[:], in_=b8_tile[:])
                b_tiles_live[kt] = b_tile

            def emit_and(kt):
                b_tile = b_tiles_live[kt]
                nc.vector.tensor_scalar(
                    out=b_tile.bitcast(mybir.dt.int32)[:],
                    in0=b_tile.bitcast(mybir.dt.int32)[:],
                    scalar1=mask_tile[:, 0:1],
                    scalar2=None,
                    op0=mybir.AluOpType.bitwise_and,
                )

            def emit_or(kt):
                b_tile = b_tiles_live.pop(kt)
                contrib = cpool.tile([P, K * OC], mybir.dt.bfloat16, name="contrib")
                nc.vector.tensor_tensor(
                    out=contrib.bitcast(mybir.dt.int16)[:],
                    in0=b_tile.bitcast(mybir.dt.int16)[:],
                    in1=a_tile.bitcast(mybir.dt.int16)[:],
                    op=mybir.AluOpType.bitwise_or,
                )
                contrib_live[kt] = contrib

            def emit_add1(kt, on_gpsimd=False):
                contrib = contrib_live.pop(kt)
                t_tile = tpool.tile([P, 2 * OC], mybir.dt.bfloat16, name="t_tile")
                eng = nc.gpsimd if on_gpsimd else nc.vector
                eng.tensor_tensor(
                    out=t_tile[:],
                    in0=contrib[:, 0:2 * OC],
                    in1=contrib[:, 2 * OC:4 * OC],
                    op=mybir.AluOpType.add,
                )
                t_live[kt] = t_tile

            def emit_add2(kt):
                t_tile = t_live.pop(kt)
                if kt < KT_BF:
                    w_tile = wpool.tile([P, OC], mybir.dt.bfloat16, name=f"w_{kt}")
                    dst = w_tile[:]
                    w_tiles[kt] = w_tile
                else:
                    g, s = divmod(kt - KT_BF, 2)
                    dst = w8_tiles[g][:, s, :]
                nc.vector.tensor_tensor(
                    out=dst,
                    in0=t_tile[:, 0:OC],
                    in1=t_tile[:, OC:2 * OC],
                    op=mybir.AluOpType.add,
                )

            def emit_build(kt, queue=None, add1_gpsimd=False):
                emit_dma(kt, queue)
                emit_and(kt)
                emit_or(kt)
                emit_add1(kt, on_gpsimd=add1_gpsimd)
                emit_add2(kt)

            # Critical path first: b0, then xt0, then the rest.
            xt_prefetch = []

            def prefetch_xt(kt):
                xt = xpool.tile([P, P * M_BLOCK], mybir.dt.bfloat16, name="xt")
                nc.sync.dma_start(out=xt[:], in_=xT[kt * P:(kt + 1) * P, 0:M_BLOCK * P])
                xt_prefetch.append(xt)

            # k-block layout: bf16 tile lists + one fp8 DR pair woven into
            # kb1..kb3 (spreads the 2x-power DR duty to ~15% everywhere so
            # HAM doesn't duty-throttle a dense fp8 phase).
            kb_tiles = [list(range(0, 7)), list(range(7, 14)),
                        list(range(14, 20)), list(range(20, 26))]
            kb_pair = [None, 0, 1, 2]
            NKB = len(kb_tiles)
            NMB = MT // M_BLOCK
            # build emission order: kb0's tiles first, then each later kb's
            # bf16 tiles followed by its pair's source tiles
            build_seq = []
            for kb in range(1, NKB):
                build_seq += kb_tiles[kb]
                g = kb_pair[kb]
                build_seq += [KT_BF + 2 * g, KT_BF + 2 * g + 1]

            emit_dma8(0)
            prefetch_xt(0)
            emit_and(0)
            emit_or(0)
            emit_add1(0, on_gpsimd=True)
            emit_add2(0)
            for kt in kb_tiles[0][1:]:
                emit_dma8(kt)
                emit_and(kt)
                emit_or(kt)
                emit_add1(kt, on_gpsimd=True)
                emit_add2(kt)
                prefetch_xt(kt)
            build_cursor = 0
            # cumulative build_seq positions to reach by the END of each kb's
            # m-loop: during kb we emit the builds needed by kb+1
            kb_build_end = [0] * NKB
            n = 0
            for kb in range(NKB - 1):
                n += len(kb_tiles[kb + 1]) + 2
                kb_build_end[kb] = n
            kb_build_end[NKB - 1] = n

            # ---- hybrid matmul ----
            import math
            acc_tiles = {}
            for kb in range(NKB):
                tiles = kb_tiles[kb]
                g = kb_pair[kb]
                start_c = kb_build_end[kb - 1] if kb else 0
                end_c = kb_build_end[kb]
                for mb in range(NMB):
                    # emit next kb's builds evenly across this kb's m-blocks
                    target = start_c + math.ceil((mb + 1) * (end_c - start_c) / NMB)
                    while build_cursor < target:
                        emit_build(build_seq[build_cursor], add1_gpsimd=(kb == 0))
                        build_cursor += 1
                    ms = [mb * M_BLOCK + j for j in range(M_BLOCK)]
                    ps_tiles = {
                        m: psum_pool.tile([P, OC], mybir.dt.float32, name=f"ps_{m % M_BLOCK}")
                        for m in ms
                    }
                    if g is not None:
                        xt8 = x8pool.tile([P, 2, P * M_BLOCK], mybir.dt.float8e4, name="xt8")
                        nc.sync.dma_start(
                            out=xt8[:, :, :],
                            in_=x8dr[g * P:(g + 1) * P, :, ms[0] * P:(ms[0] + M_BLOCK) * P],
                        )
                    for kt in tiles:
                        if kb == 0 and mb == 0:
                            xt = xt_prefetch[kb_tiles[0].index(kt)]
                        else:
                            xt = xpool.tile([P, P * M_BLOCK], mybir.dt.bfloat16, name="xt")
                            nc.sync.dma_start(
                                out=xt[:],
                                in_=xT[kt * P:(kt + 1) * P,
                                      ms[0] * P:(ms[0] + M_BLOCK) * P],
                            )
                        for j, m in enumerate(ms):
                            nc.tensor.matmul(
                                ps_tiles[m][:],
                                xt[:, j * P:(j + 1) * P],
                                w_tiles[kt][:],
                                start=(kt == tiles[0]),
                                stop=(kt == tiles[-1] and g is None),
                            )
                    if g is not None:
                        for j, m in enumerate(ms):
                            nc.tensor.matmul(
                                ps_tiles[m][:],
                                xt8[:, :, j * P:(j + 1) * P],
                                w8_tiles[g][:, :, :],
                                start=False,
                                stop=True,
                                perf_mode=mybir.MatmulPerfMode.DoubleRow,
                            )
                    for m in ms:
                        if kb == 0:
                            acc = apool.tile([P, OC], mybir.dt.float32, name=f"acc_{m}")
                            nc.vector.tensor_tensor(
                                out=acc[:], in0=ps_tiles[m][:], in1=bias_tile[:],
                                op=mybir.AluOpType.add,
                            )
                            acc_tiles[m] = acc
                        elif kb < NKB - 1:
                            # build still running on DVE: route via scalar+gpsimd
                            acc = acc_tiles[m]
                            tmp = eapool.tile([P, OC], mybir.dt.float32, name="ea")
                            nc.scalar.copy(out=tmp[:], in_=ps_tiles[m][:])
                            nc.gpsimd.tensor_tensor(
                                out=acc[:], in0=tmp[:], in1=acc[:],
                                op=mybir.AluOpType.add,
                            )
                        elif kb == NKB - 1:
                            # build done: DVE reads psum directly; gpsimd ships
                            acc = acc_tiles[m]
                            o_tile = eapool.tile([P, OC], mybir.dt.float32, name="ea")
                            nc.vector.tensor_tensor(
                                out=o_tile[:], in0=ps_tiles[m][:], in1=acc[:],
                                op=mybir.AluOpType.add,
                            )
                            nc.gpsimd.dma_start(out=out[m * P:(m + 1) * P, :], in_=o_tile[:])

    nc.compile()
    return nc


_NC_CACHE = None


def _get_program():
    global _NC_CACHE
    if _NC_CACHE is None:
        _NC_CACHE = _build_program()
    return _NC_CACHE


def prep_inputs(x, b, a, bias):
    """Host-side shard/cast/layout only. Returns per-core input maps."""
    x = np.asarray(x, dtype=np.float32)
    b = np.asarray(b, dtype=np.float32)
    a = np.asarray(a, dtype=np.float32)
    bias = np.asarray(bias, dtype=np.float32)
    x16 = x.astype(BF16)
    xT16 = np.ascontiguousarray(x16.T)                      # [I, B] bf16
    # fp8 pair layout for k-tiles 26..31: [3*128, 2, B]
    x8T = np.ascontiguousarray(x16.astype(E4M3).T)          # [I, B] e4m3
    v = x8T.reshape(KT, P, B)
    x8dr = np.ascontiguousarray(
        np.stack([np.stack([v[KT_BF + 2 * g], v[KT_BF + 2 * g + 1]], axis=1)
                  for g in range(N_PAIR)], axis=0).reshape(N_PAIR * P, 2, B))
    b_iko = np.transpose(b, (1, 0, 2)).astype(BF16)         # [I, K, O] bf16
    bias32 = bias.astype(np.float32)
    a16 = a.astype(BF16)                                     # [K, O]

    in_maps = []
    for c in range(NCORES):
        sl = slice(c * OC, (c + 1) * OC)
        b_slice = np.ascontiguousarray(b_iko[:, :, sl]).reshape(I, K * OC)
        b8_slice = b_slice[0:7 * P].astype(E4M3)
        a_flat = np.ascontiguousarray(a16[:, sl]).reshape(1, K * OC)
        a_bcast = np.broadcast_to(a_flat, (P, K * OC)).copy()
        bias_bcast = np.broadcast_to(bias32[sl].reshape(1, OC), (P, OC)).copy()
        in_maps.append({
            "b_re": b_slice,
            "b8_re": b8_slice,
            "a_b": a_bcast,
            "xT": xT16,
            "x8dr": x8dr,
            "bias_b": bias_bcast,
        })
    return in_maps


def run(in_maps, trace=False):
    from concourse.bass_utils import run_bass_kernel_spmd

    nc = _get_program()
    res = run_bass_kernel_spmd(nc, in_maps, list(range(NCORES)), trace=trace)
    return res


def kernel(x, b, a, bias):
    in_maps = prep_inputs(x, b, a, bias)
    res = run(in_maps)
    out = np.concatenate([res.results[c]["out"] for c in range(NCORES)], axis=1)
    return np.ascontiguousarray(out, dtype=np.float32)


if __name__ == "__main__":
    rng = np.random.default_rng(0)
    x = rng.standard_normal((B, I), dtype=np.float32)
    b = rng.standard_normal((K, I, O), dtype=np.float32)
    a = rng.random((K, O), dtype=np.float32)
    bias = rng.standard_normal(O, dtype=np.float32)
    out = kernel(x=x, b=b, a=a, bias=bias)
    w_eff = np.einsum('kio,ko->io', np.sign(b), a.astype(np.float64)).astype(np.float64)
    expected = x.astype(np.float64) @ w_eff + bias
    rel = np.linalg.norm(out - expected) / np.linalg.norm(expected)
    print(f"rel_err = {rel:.3e}")
